# revision 5
# baseline (speedup 1.0000x reference)
"""Causal self-attention (B=4, T=2048, D=1024, H=16) on 8 Trainium2 NeuronCores.

Sharding: batch x head-half. Core c handles batch b = c//2 and heads
hh..hh+7 where hh = 8*(c%2)  (tensor-parallel split of w_qkv output dim and
w_o input dim). Each core produces a partial o_proj output [2048, 1024];
the host sums the two partials per batch (the 2-way all-reduce).

Per-core kernel (all matmuls bf16, fp32 PSUM accumulate):
  phase 1: qkv projection. Q^T,K^T produced head-pair-stacked [128, t] for
           row-tiled score matmuls; V produced in natural [t, dk] layout with
           an appended ones column (row-sum trick).
  phase 2: flash-style causal attention per (q-tile of 512, head-pair):
           S^T = K^T.T @ Q^T via two row-tiled (K=64) matmuls, exp on ACT
           (scale 1/8 folded in), causal masking on diagonal chunks via a
           static mask multiply, PV+rowsum via [128,65] stationary matmuls
           accumulating in PSUM, then normalize by 1/rowsum.
  phase 3: o_proj partial = out_heads^T.T @ w_o^T slice (overlaps phase 2
           since q-tiles complete incrementally).
"""
import numpy as np
import ml_dtypes

B, T, D, H = 4, 2048, 1024, 16
DK = D // H          # 64
HPC = 8              # heads per core
NCORES = 8
NQT = T // 512       # 4
NKC = T // 128       # 16

_cache = {}


def _emit(nc, tc, pools, dram):
    import concourse.mybir as mybir

    bf16 = mybir.dt.bfloat16
    f32 = mybir.dt.float32
    Exp = mybir.ActivationFunctionType.Exp
    cst, big, work, stp, psa, psb = pools
    xt_d, wq_d, wo_d, out_d, masks = dram

    # per-d-chunk input tiles so compute can start before the full load
    xts = [big.tile([128, T], bf16, tag=f"xt{dc}", name=f"xts{dc}") for dc in range(8)]
    wqs = [big.tile([128, 1536], bf16, tag=f"wq{dc}", name=f"wqs{dc}") for dc in range(8)]
    wos = [big.tile([128, D], bf16, tag=f"wo{pr}", name=f"wos{pr}") for pr in range(4)]
    qk = big.tile([128, 8, T], bf16, tag="qk")
    vt = big.tile([128, NKC, HPC, DK + 1], bf16, tag="vt")
    ob = big.tile([128, 4, T], bf16, tag="ob")

    for dc in range(8):
        nc.gpsimd.dma_start(xts[dc][:], xt_d[dc])
        nc.gpsimd.dma_start(wqs[dc][:], wq_d[dc])
    for pr in range(4):
        nc.gpsimd.dma_start(wos[pr][:], wo_d[pr])

    nc.gpsimd.memset(vt[:, :, :, DK], 1.0)

    # ---- phase 1a: Q^T / K^T  (head-pair-stacked chunks) ----
    for ec in range(8):
        for tcx in range(NQT):
            ps = psa.tile([128, 512], f32, tag="s")
            for dc in range(8):
                nc.tensor.matmul(
                    ps[:],
                    wqs[dc][:, ec * 128:(ec + 1) * 128],
                    xts[dc][:, tcx * 512:(tcx + 1) * 512],
                    start=(dc == 0), stop=(dc == 7),
                )
            nc.vector.tensor_copy(qk[:, ec, tcx * 512:(tcx + 1) * 512], ps[:])

    # ---- phase 1b: V (natural layout, scattered by head) ----
    for tt in range(NKC):
        ps = psa.tile([128, 512], f32, tag="s")
        for dc in range(8):
            nc.tensor.matmul(
                ps[:],
                xts[dc][:, tt * 128:(tt + 1) * 128],
                wqs[dc][:, 1024:1536],
                start=(dc == 0), stop=(dc == 7),
            )
        nc.vector.tensor_copy(
            vt[:, tt, :, 0:DK], ps[:].rearrange("p (h d) -> p h d", d=DK))

    # ---- phase 2: causal attention (qt outer so o_proj can overlap) ----
    for qt in range(NQT):
        for pr in range(4):          # head pairs (2*pr, 2*pr+1)
            pv0 = psb.tile([65, 512], f32, tag="pv")
            pv1 = psb.tile([65, 512], f32, tag="pv")
            nkc = 4 * qt + 4
            for kc in range(nkc):
                s0 = psa.tile([128, 512], f32, tag="s")
                s1 = psa.tile([128, 512], f32, tag="s")
                nc.tensor.matmul(
                    s0[:], qk[0:64, 4 + pr, kc * 128:(kc + 1) * 128],
                    qk[0:64, pr, qt * 512:(qt + 1) * 512],
                    start=True, stop=True, tile_position=(0, 0))
                nc.tensor.matmul(
                    s1[:], qk[64:128, 4 + pr, kc * 128:(kc + 1) * 128],
                    qk[64:128, pr, qt * 512:(qt + 1) * 512],
                    start=True, stop=True, tile_position=(64, 0))
                st0 = stp.tile([128, 512], bf16, tag="st")
                st1 = stp.tile([128, 512], bf16, tag="st")
                i = kc - 4 * qt
                lo = max(i, 0) * 128   # first unmasked column of this chunk
                nc.scalar.activation(st0[:, lo:], s0[:, lo:], Exp, scale=0.125)
                nc.scalar.activation(st1[:, lo:], s1[:, lo:], Exp, scale=0.125)
                if i >= 0:      # diagonal chunk: mask the triangular block
                    nc.vector.tensor_mul(
                        st0[:, lo:lo + 128], st0[:, lo:lo + 128],
                        masks[:])
                    nc.vector.tensor_mul(
                        st1[:, lo:lo + 128], st1[:, lo:lo + 128],
                        masks[:])
                nc.tensor.matmul(
                    pv0[:, lo:], vt[:, kc, 2 * pr, :], st0[:, lo:],
                    start=(kc == 0), stop=(kc == nkc - 1))
                nc.tensor.matmul(
                    pv1[:, lo:], vt[:, kc, 2 * pr + 1, :], st1[:, lo:],
                    start=(kc == 0), stop=(kc == nkc - 1))
            # normalize: ob[h-part, pr, qt] = pv[0:64] * (1/rowsum)
            for hh, pv in ((0, pv0), (1, pv1)):
                # custom-DVE ops ignore the input AP's partition base, so
                # stage the sums row at partition 0 first
                sd = work.tile([1, 512], f32, tag="sd")
                nc.vector.tensor_copy(sd[:], pv[64:65, :])
                rc = work.tile([1, 512], f32, tag="rc")
                nc.vector.reciprocal_approx_fast(rc[:], sd[:])
                rb = work.tile([64, 512], f32, tag="rb")
                nc.gpsimd.partition_broadcast(rb[:], rc[:])
                nc.vector.tensor_mul(
                    ob[64 * hh:64 * hh + 64, pr, qt * 512:(qt + 1) * 512],
                    pv[0:64, :], rb[:])

        # ---- phase 3 (interleaved): o_proj for the finished q-range ----
        for tt in range(4 * qt, 4 * qt + 4):
            for eh in range(2):
                po = psb.tile([128, 512], f32, tag="pv")
                for pr in range(4):
                    nc.tensor.matmul(
                        po[:], ob[:, pr, tt * 128:(tt + 1) * 128],
                        wos[pr][:, eh * 512:(eh + 1) * 512],
                        start=(pr == 0), stop=(pr == 3))
                ot = work.tile([128, 512], f32, tag="ot")
                nc.vector.tensor_copy(ot[:], po[:])
                nc.gpsimd.dma_start(
                    out_d[tt * 128:(tt + 1) * 128,
                          eh * 512:(eh + 1) * 512], ot[:])


def _build(reps=1):
    import concourse.mybir as mybir
    import concourse.tile as tile
    from concourse import bacc

    bf16 = mybir.dt.bfloat16
    f32 = mybir.dt.float32

    nc = bacc.Bacc("TRN2", target_bir_lowering=False, debug=False,
                   num_devices=NCORES)
    xt_d = nc.dram_tensor("xt", [8, 128, T], bf16, kind="ExternalInput")
    wq_d = nc.dram_tensor("wq", [8, 128, 1536], bf16, kind="ExternalInput")
    wo_d = nc.dram_tensor("wo", [4, 128, D], bf16, kind="ExternalInput")
    out_d = nc.dram_tensor("out", [T, D], f32, kind="ExternalOutput")

    with tile.TileContext(nc) as tc:
        with (
            tc.tile_pool(name="cst", bufs=1) as cst,
            tc.tile_pool(name="big", bufs=1) as big,
            tc.tile_pool(name="work", bufs=4) as work,
            tc.tile_pool(name="stp", bufs=6) as stp,
            tc.tile_pool(name="psa", bufs=4, space="PSUM") as psa,
            tc.tile_pool(name="psb", bufs=4, space="PSUM") as psb,
        ):
            # static causal mask for the 128x128 diagonal blocks:
            # masks[p, q] = 1 if q >= p else 0
            masks = cst.tile([128, 128], bf16)
            nc.gpsimd.memset(masks[:], 1.0)
            nc.gpsimd.affine_select(
                out=masks[:], in_=masks[:],
                compare_op=mybir.AluOpType.is_ge, fill=0.0,
                base=0, channel_multiplier=-1, pattern=[[1, 128]],
            )
            pools = (cst, big, work, stp, psa, psb)
            dram = (xt_d, wq_d, wo_d, out_d, masks)
            if reps == 1:
                _emit(nc, tc, pools, dram)
            else:
                with tc.For_i(0, reps, 1):
                    _emit(nc, tc, pools, dram)

    nc.compile()
    return nc


def prep_inputs(x, w_qkv, w_o):
    """Host-side shard + layout prep. Returns in_maps for cores 0..7."""
    bf = ml_dtypes.bfloat16
    in_maps = []
    for c in range(NCORES):
        b, hh = c // 2, HPC * (c % 2)
        qrows = w_qkv[hh * DK:(hh + HPC) * DK]                    # [512, 1024]
        krows = w_qkv[D + hh * DK:D + (hh + HPC) * DK]
        vrows = w_qkv[2 * D + hh * DK:2 * D + (hh + HPC) * DK]
        wqt = np.concatenate([qrows, krows, vrows], 0).T          # [1024, 1536]
        in_maps.append({
            "xt": np.ascontiguousarray(x[b].T).astype(bf).reshape(8, 128, T),
            "wq": wqt.astype(bf).reshape(8, 128, 1536),
            "wo": np.ascontiguousarray(w_o[:, hh * DK:(hh + HPC) * DK].T)
                    .astype(bf).reshape(4, 128, D),
        })
    return in_maps


def get_nc(reps=1):
    key = ("nc", reps)
    if key not in _cache:
        _cache[key] = _build(reps)
    return _cache[key]


def kernel(x, w_qkv, w_o):
    from concourse.bass_utils import run_bass_kernel_spmd

    nc = get_nc()
    in_maps = prep_inputs(np.asarray(x, dtype=np.float32),
                          np.asarray(w_qkv, dtype=np.float32),
                          np.asarray(w_o, dtype=np.float32))
    res = run_bass_kernel_spmd(nc, in_maps, core_ids=list(range(NCORES)))
    out = np.empty((B, T, D), np.float32)
    for b in range(B):
        out[b] = res.results[2 * b]["out"] + res.results[2 * b + 1]["out"]
    return out


# revision 6
# speedup vs baseline: 1.4140x; 1.4140x over previous
"""Causal self-attention (B=4, T=2048, D=1024, H=16) on 8 Trainium2 NeuronCores.

Sharding: batch x head-half. Core c handles batch b = c//2 and heads
hh..hh+7 where hh = 8*(c%2)  (tensor-parallel split of w_qkv output dim and
w_o input dim). Each core produces a partial o_proj output [2048, 1024];
the host sums the two partials per batch (the 2-way all-reduce).

Per-core kernel (all matmuls bf16, fp32 PSUM accumulate):
  phase 1: qkv projection. Q^T,K^T produced head-pair-stacked [128, t] for
           row-tiled score matmuls; V produced in natural [t, dk] layout with
           an appended ones column (row-sum trick).
  phase 2: flash-style causal attention per (q-tile of 512, head-pair):
           S^T = K^T.T @ Q^T via two row-tiled (K=64) matmuls, exp on ACT
           (scale 1/8 folded in), causal masking on diagonal chunks via a
           static mask multiply, PV+rowsum via [128,65] stationary matmuls
           accumulating in PSUM, then normalize by 1/rowsum.
  phase 3: o_proj partial = out_heads^T.T @ w_o^T slice (overlaps phase 2
           since q-tiles complete incrementally).
"""
import numpy as np
import ml_dtypes

B, T, D, H = 4, 2048, 1024, 16
DK = D // H          # 64
HPC = 8              # heads per core
NCORES = 8
NQT = T // 512       # 4
NKC = T // 128       # 16

_cache = {}


def _emit(nc, tc, pools, dram):
    import concourse.mybir as mybir

    bf16 = mybir.dt.bfloat16
    f32 = mybir.dt.float32
    Exp = mybir.ActivationFunctionType.Exp
    cst, big, work, stp, psa, psb = pools
    xt_d, wq_d, wo_d, out_d, masks = dram

    # per-d-chunk input tiles so compute can start before the full load
    xts = [big.tile([128, T], bf16, tag=f"xt{dc}", name=f"xts{dc}") for dc in range(8)]
    wqs = [big.tile([128, 1536], bf16, tag=f"wq{dc}", name=f"wqs{dc}") for dc in range(8)]
    wos = [big.tile([128, D], bf16, tag=f"wo{pr}", name=f"wos{pr}") for pr in range(4)]
    qk = big.tile([128, 8, T], bf16, tag="qk")
    vt = big.tile([128, NKC, HPC, DK + 1], bf16, tag="vt")
    ob = big.tile([128, 4, T], bf16, tag="ob")

    for dc in range(8):
        nc.gpsimd.dma_start(xts[dc][:], xt_d[dc])
        nc.gpsimd.dma_start(wqs[dc][:], wq_d[dc])
    for pr in range(4):
        nc.gpsimd.dma_start(wos[pr][:], wo_d[pr])

    nc.gpsimd.memset(vt[:, :, :, DK], 1.0)

    # ---- phase 1a: Q^T / K^T  (head-pair-stacked chunks) ----
    for ec in range(8):
        for tcx in range(NQT):
            ps = psa.tile([128, 512], f32, tag="s")
            for dc in range(8):
                nc.tensor.matmul(
                    ps[:],
                    wqs[dc][:, ec * 128:(ec + 1) * 128],
                    xts[dc][:, tcx * 512:(tcx + 1) * 512],
                    start=(dc == 0), stop=(dc == 7),
                )
            nc.vector.tensor_copy(qk[:, ec, tcx * 512:(tcx + 1) * 512], ps[:])

    # ---- phase 1b: V (natural layout, scattered by head) ----
    for tt in range(NKC):
        ps = psa.tile([128, 512], f32, tag="s")
        for dc in range(8):
            nc.tensor.matmul(
                ps[:],
                xts[dc][:, tt * 128:(tt + 1) * 128],
                wqs[dc][:, 1024:1536],
                start=(dc == 0), stop=(dc == 7),
            )
        nc.vector.tensor_copy(
            vt[:, tt, :, 0:DK], ps[:].rearrange("p (h d) -> p h d", d=DK))

    # ---- phase 2: causal attention (qt outer so o_proj can overlap) ----
    for qt in range(NQT):
        for pr in range(4):          # head pairs (2*pr, 2*pr+1)
            pv0 = psb.tile([65, 512], f32, tag="pv")
            pv1 = psb.tile([65, 512], f32, tag="pv")
            nkc = 4 * qt + 4
            for kc in range(nkc):
                i = kc - 4 * qt
                lo = max(i, 0) * 128   # first unmasked column of this chunk
                s0 = psa.tile([128, 512], f32, tag="s")
                s1 = psa.tile([128, 512], f32, tag="s")
                nc.tensor.matmul(
                    s0[:, lo:], qk[0:64, 4 + pr, kc * 128:(kc + 1) * 128],
                    qk[0:64, pr, qt * 512 + lo:(qt + 1) * 512],
                    start=True, stop=True, tile_position=(0, 0))
                nc.tensor.matmul(
                    s1[:, lo:], qk[64:128, 4 + pr, kc * 128:(kc + 1) * 128],
                    qk[64:128, pr, qt * 512 + lo:(qt + 1) * 512],
                    start=True, stop=True, tile_position=(64, 0))
                st0 = stp.tile([128, 512], bf16, tag="st")
                st1 = stp.tile([128, 512], bf16, tag="st")
                nc.scalar.activation(st0[:, lo:], s0[:, lo:], Exp, scale=0.125)
                nc.scalar.activation(st1[:, lo:], s1[:, lo:], Exp, scale=0.125)
                if i >= 0:      # diagonal chunk: mask the triangular block
                    nc.vector.tensor_mul(
                        st0[:, lo:lo + 128], st0[:, lo:lo + 128],
                        masks[:])
                    nc.vector.tensor_mul(
                        st1[:, lo:lo + 128], st1[:, lo:lo + 128],
                        masks[:])
                nc.tensor.matmul(
                    pv0[:, lo:], vt[:, kc, 2 * pr, :], st0[:, lo:],
                    start=(kc == 0), stop=(kc == nkc - 1))
                nc.tensor.matmul(
                    pv1[:, lo:], vt[:, kc, 2 * pr + 1, :], st1[:, lo:],
                    start=(kc == 0), stop=(kc == nkc - 1))
            # normalize: ob[h-part, pr, qt] = pv[0:64] * (1/rowsum)
            for hh, pv in ((0, pv0), (1, pv1)):
                # custom-DVE ops ignore the input AP's partition base, so
                # stage the sums row at partition 0 first
                sd = work.tile([1, 512], f32, tag="sd")
                nc.vector.tensor_copy(sd[:], pv[64:65, :])
                rc = work.tile([1, 512], f32, tag="rc")
                nc.vector.reciprocal_approx_fast(rc[:], sd[:])
                rb = work.tile([64, 512], f32, tag="rb")
                nc.gpsimd.partition_broadcast(rb[:], rc[:])
                nc.vector.tensor_mul(
                    ob[64 * hh:64 * hh + 64, pr, qt * 512:(qt + 1) * 512],
                    pv[0:64, :], rb[:])

        # ---- phase 3 (interleaved): o_proj for the finished q-range ----
        for tt in range(4 * qt, 4 * qt + 4):
            for eh in range(2):
                po = psb.tile([128, 512], f32, tag="pv")
                for pr in range(4):
                    nc.tensor.matmul(
                        po[:], ob[:, pr, tt * 128:(tt + 1) * 128],
                        wos[pr][:, eh * 512:(eh + 1) * 512],
                        start=(pr == 0), stop=(pr == 3))
                ot = work.tile([128, 512], f32, tag="ot")
                nc.vector.tensor_copy(ot[:], po[:])
                nc.gpsimd.dma_start(
                    out_d[tt * 128:(tt + 1) * 128,
                          eh * 512:(eh + 1) * 512], ot[:])


def _build(reps=1):
    import concourse.mybir as mybir
    import concourse.tile as tile
    from concourse import bacc

    bf16 = mybir.dt.bfloat16
    f32 = mybir.dt.float32

    nc = bacc.Bacc("TRN2", target_bir_lowering=False, debug=False,
                   num_devices=NCORES)
    xt_d = nc.dram_tensor("xt", [8, 128, T], bf16, kind="ExternalInput")
    wq_d = nc.dram_tensor("wq", [8, 128, 1536], bf16, kind="ExternalInput")
    wo_d = nc.dram_tensor("wo", [4, 128, D], bf16, kind="ExternalInput")
    out_d = nc.dram_tensor("out", [T, D], f32, kind="ExternalOutput")

    with tile.TileContext(nc) as tc:
        with (
            tc.tile_pool(name="cst", bufs=1) as cst,
            tc.tile_pool(name="big", bufs=1) as big,
            tc.tile_pool(name="work", bufs=4) as work,
            tc.tile_pool(name="stp", bufs=6) as stp,
            tc.tile_pool(name="psa", bufs=4, space="PSUM") as psa,
            tc.tile_pool(name="psb", bufs=4, space="PSUM") as psb,
        ):
            # static causal mask for the 128x128 diagonal blocks:
            # masks[p, q] = 1 if q >= p else 0
            masks = cst.tile([128, 128], bf16)
            nc.gpsimd.memset(masks[:], 1.0)
            nc.gpsimd.affine_select(
                out=masks[:], in_=masks[:],
                compare_op=mybir.AluOpType.is_ge, fill=0.0,
                base=0, channel_multiplier=-1, pattern=[[1, 128]],
            )
            pools = (cst, big, work, stp, psa, psb)
            dram = (xt_d, wq_d, wo_d, out_d, masks)
            if reps == 1:
                _emit(nc, tc, pools, dram)
            else:
                with tc.For_i(0, reps, 1):
                    _emit(nc, tc, pools, dram)

    nc.compile()
    return nc


def prep_inputs(x, w_qkv, w_o):
    """Host-side shard + layout prep. Returns in_maps for cores 0..7."""
    bf = ml_dtypes.bfloat16
    in_maps = []
    for c in range(NCORES):
        b, hh = c // 2, HPC * (c % 2)
        qrows = w_qkv[hh * DK:(hh + HPC) * DK]                    # [512, 1024]
        krows = w_qkv[D + hh * DK:D + (hh + HPC) * DK]
        vrows = w_qkv[2 * D + hh * DK:2 * D + (hh + HPC) * DK]
        wqt = np.concatenate([qrows, krows, vrows], 0).T          # [1024, 1536]
        in_maps.append({
            "xt": np.ascontiguousarray(x[b].T).astype(bf).reshape(8, 128, T),
            "wq": wqt.astype(bf).reshape(8, 128, 1536),
            "wo": np.ascontiguousarray(w_o[:, hh * DK:(hh + HPC) * DK].T)
                    .astype(bf).reshape(4, 128, D),
        })
    return in_maps


def get_nc(reps=1):
    key = ("nc", reps)
    if key not in _cache:
        _cache[key] = _build(reps)
    return _cache[key]


def kernel(x, w_qkv, w_o):
    from concourse.bass_utils import run_bass_kernel_spmd

    nc = get_nc()
    in_maps = prep_inputs(np.asarray(x, dtype=np.float32),
                          np.asarray(w_qkv, dtype=np.float32),
                          np.asarray(w_o, dtype=np.float32))
    res = run_bass_kernel_spmd(nc, in_maps, core_ids=list(range(NCORES)))
    out = np.empty((B, T, D), np.float32)
    for b in range(B):
        out[b] = res.results[2 * b]["out"] + res.results[2 * b + 1]["out"]
    return out
